# revision 29
# baseline (speedup 1.0000x reference)
"""Single-head causal attention (B=1024,T=256,C=512,H=64), data-parallel on 8 TRN2 cores.

Host side: cast x to bf16; pack Wq|Wk into a [128, 4*128] stationary layout
(chunk j at cols 128j, cols 0:64 = Wq chunk, 64:128 = Wk chunk) and Wv into
[128, 4*64]. Device output is [nb/8, 128, 1024] f32 in a permuted layout that
the host untangles (big contiguous HBM lines for the store DMA).

Device side, per pair of batches (b0, b0+1):
  xt [128, 2048] bf16 = xT chunks, layout j*512 + b*256 + t, filled by 8
      XBAR transpose-DMAs straight from HBM (no PE transposes, no casts)
  qkT [128 (q|k), 512 (b,t)] = Wqk^T @ xT   (4 matmuls, N=512)
  v   [128 (s), 4*64 (b,st)] natural, xT chunks stationary (16 small matmuls)
  v1  [128, 130] per batch = [v | 1] per s-tile (ones col -> softmax denom)
  weiT/exp: w_ps [128 (s), 384 (t-tile0 256 | diag1 128)], exp via one
      scalar activation, causal mask via 2 affine_selects on diagonal blocks
  out natural [t, 65] = sum_st e_chunk^T @ v1_st; normalize by col 64
"""

import sys, json

for _p in ("/opt/trn_rl_repo", "/root/.axon_site/_ro/trn_rl_repo"):
    if _p not in sys.path:
        sys.path.append(_p)

import numpy as np
import concourse.bass as bass
import concourse.tile as tile
from concourse import mybir
from concourse.bass_utils import run_bass_kernel_spmd

N_CORES = 8
B, T, C, H = 1024, 256, 512, 64
NB = B // N_CORES  # batches per core
CD = mybir.dt.bfloat16
F32 = mybir.dt.float32

_MAX_CTRL_WAITS = 1


def _patch_waits(nc):
    """walrus on this toolchain rejects >1 sync-wait on TPB_CTRL (NoOp/Drain/
    EventSemaphore) instructions; hoist excess waits into preceding NoOps."""
    raw = type(nc).to_json_bytes(nc)
    j = json.loads(raw)
    ctr = 0
    for f in j.get("functions", []):
        for bb in f.get("basicblocks", f.get("blocks", [])):
            out = []
            for i in bb.get("instructions", []):
                si = i.get("sync_info") or {}
                ow = si.get("on_wait") or []
                has_update = bool((si.get("on_update") or []))
                splittable = i.get("opcode") != "EventSemaphore" or not has_update
                if len(ow) > _MAX_CTRL_WAITS and splittable:
                    excess, keep = ow[:-_MAX_CTRL_WAITS], ow[-_MAX_CTRL_WAITS:]
                    while excess:
                        chunk, excess = excess[:_MAX_CTRL_WAITS], excess[_MAX_CTRL_WAITS:]
                        ctr += 1
                        out.append({
                            "name": f"WSPLIT-{ctr}",
                            "opcode": "NoOp",
                            "engine": i["engine"],
                            "ins": [], "outs": [],
                            "debug": i.get("debug", 0),
                            "sync_info": {"on_wait": chunk, "on_update": []},
                        })
                    si["on_wait"] = keep
                    i["sync_info"] = si
                out.append(i)
            bb["instructions"] = out
    data = json.dumps(j).encode()
    nc.to_json_bytes = lambda: data
    return nc


def build(nb=NB):
    assert nb % 2 == 0
    nc = bass.Bass("TRN2", target_bir_lowering=False, debug=False, enable_asserts=False)
    # host-packed xT: [pair, c-in-chunk, (j, b, t)] — 4KB contiguous HBM lines
    x = nc.dram_tensor("x", [nb // 2, 128, 2048], CD, kind="ExternalInput").ap()
    wqk_d = nc.dram_tensor("Wqk", [128, 512], CD, kind="ExternalInput").ap()
    wv_d = nc.dram_tensor("Wv", [128, 256], CD, kind="ExternalInput").ap()
    y = nc.dram_tensor("y", [nb // 2, 128, 256], CD, kind="ExternalOutput").ap()

    with tile.TileContext(nc) as tc:
        with (
            tc.tile_pool(name="consts", bufs=1) as consts,
            tc.tile_pool(name="xt", bufs=5) as p_xt,
            tc.tile_pool(name="qk", bufs=3) as p_qk,
            tc.tile_pool(name="v1", bufs=6) as p_v1,
            tc.tile_pool(name="e", bufs=6) as p_e,
            tc.tile_pool(name="rcp", bufs=4) as p_rcp,
            tc.tile_pool(name="go", bufs=4) as p_go,
            tc.tile_pool(name="ps", bufs=2, space="PSUM") as p_ps,
        ):
            wqk = consts.tile([128, 512], CD)
            nc.sync.dma_start(wqk[:], wqk_d[:, :])
            wvs = consts.tile([128, 256], CD)
            nc.sync.dma_start(wvs[:], wv_d[:, :])

            def emit_tail(prev):
                """out matmuls + normalize + store for an earlier pair (deps long ready)."""
                gout = p_go.tile([128, 256], CD, tag="gout", name="gout")
                for b in range(2):
                    e, v1 = prev["e"][b], prev["v1"][b]
                    o_ps = p_ps.tile([128, 130], F32, tag="o_ps", name="o_ps")
                    nc.tensor.matmul(
                        o_ps[:, 0:65], e[:, 0:128], v1[:, 0:65],
                        start=True, stop=True, skip_group_check=True,
                    )
                    nc.tensor.matmul(
                        o_ps[:, 65:130], e[:, 128:256], v1[:, 0:65],
                        start=True, stop=False, skip_group_check=True,
                    )
                    nc.tensor.matmul(
                        o_ps[:, 65:130], e[:, 256:384], v1[:, 65:130],
                        start=False, stop=True, skip_group_check=True,
                    )
                    rcp = p_rcp.tile([128, 2], F32, tag="rcp", name="rcp")
                    nc.vector.reciprocal(rcp[:, 0:1], o_ps[:, 64:65])
                    nc.vector.reciprocal(rcp[:, 1:2], o_ps[:, 129:130])
                    nc.scalar.mul(
                        gout[:, b * 128 : b * 128 + 64], o_ps[:, 0:64], rcp[:, 0:1]
                    )
                    nc.vector.tensor_scalar_mul(
                        gout[:, b * 128 + 64 : b * 128 + 128], o_ps[:, 65:129], rcp[:, 1:2]
                    )
                nc.sync.dma_start(y[prev["pair"]], gout[:])

            prev = None
            for pair in range(nb // 2):
                # ---- xT tile [128 (c-in-chunk), (j, b, t)] straight from HBM ----
                xt = p_xt.tile([128, 2048], CD, tag="xt", name="xt")
                nc.sync.dma_start(xt[:], x[pair])

                # ---- qkT [128 (q0:64|k64:128), 512 (b,t)] ----
                qk_ps = p_ps.tile([128, 512], F32, tag="qk_ps", name="qk_ps")
                for j in range(4):
                    nc.tensor.matmul(
                        qk_ps[:],
                        wqk[:, 128 * j : 128 * (j + 1)],
                        xt[:, j * 512 : (j + 1) * 512],
                        start=(j == 0), stop=(j == 3),
                    )
                qt = p_qk.tile([64, 512], CD, tag="qt", name="qt")
                nc.vector.tensor_copy(qt[:], qk_ps[0:64, :])
                kt = p_qk.tile([64, 512], CD, tag="kt", name="kt")
                nc.vector.tensor_copy(kt[:], qk_ps[64:128, :])

                # ---- previous pair's output stage fills PE while exp/mask runs ----
                if prev is not None:
                    emit_tail(prev)

                # ---- v natural, pre-gapped [v|1] blocks: g at cols 65g ----
                v_ps = p_ps.tile([128, 260], F32, tag="v_ps", name="v_ps")
                for g in range(4):
                    nc.vector.memset(v_ps[:, g * 65 + 64 : g * 65 + 65], 1.0)
                for b in range(2):
                    for st in range(2):
                        g = b * 2 + st
                        for j in range(4):
                            nc.tensor.matmul(
                                v_ps[:, g * 65 : g * 65 + 64],
                                xt[:, j * 512 + b * 256 + st * 128 : j * 512 + b * 256 + st * 128 + 128],
                                wvs[:, 64 * j : 64 * (j + 1)],
                                start=(j == 0), stop=(j == 3),
                                skip_group_check=True,
                            )

                cur = {"pair": pair, "e": [], "v1": []}
                for b in range(2):
                    # ---- v1 [128, 130]: [v_s0 | 1 | v_s1 | 1], single copy ----
                    v1 = p_v1.tile([128, 130], CD, tag="v1", name="v1")
                    if b == 0:
                        nc.scalar.copy(v1[:], v_ps[:, 0:130])
                    else:
                        nc.vector.tensor_copy(v1[:], v_ps[:, 130:260])

                    # ---- weiT: [s0, t 0:256 | s1, t 128:256] ----
                    w_ps = p_ps.tile([128, 384], F32, tag="w_ps", name="w_ps")
                    nc.tensor.matmul(
                        w_ps[:, 0:256],
                        kt[:, b * 256 : b * 256 + 128],
                        qt[:, b * 256 : (b + 1) * 256],
                        start=True, stop=True, skip_group_check=True,
                    )
                    nc.tensor.matmul(
                        w_ps[:, 256:384],
                        kt[:, b * 256 + 128 : (b + 1) * 256],
                        qt[:, b * 256 + 128 : (b + 1) * 256],
                        start=True, stop=True, skip_group_check=True,
                    )
                    e = p_e.tile([128, 384], CD, tag="e", name="e")
                    nc.scalar.activation(e[:], w_ps[:], mybir.ActivationFunctionType.Exp, scale=0.125)
                    # causal mask only on the diagonal blocks (t>=s kept)
                    for off in (0, 256):
                        nc.gpsimd.affine_select(
                            out=e[:, off : off + 128], in_=e[:, off : off + 128],
                            compare_op=mybir.AluOpType.is_ge,
                            fill=0.0, base=0, pattern=[[1, 128]], channel_multiplier=-1,
                        )
                    cur["e"].append(e)
                    cur["v1"].append(v1)
                prev = cur

            emit_tail(prev)

    return _patch_waits(nc)


_CACHED = {}


def _get_nc(nb=NB):
    if nb not in _CACHED:
        _CACHED[nb] = build(nb)
    return _CACHED[nb]


_PACK_JIT = None


def _pack_x(x):
    """[B, 256, 512] f32 -> [B//2 pairs, 128, (j,b,t)] bf16, xT chunk layout."""
    global _PACK_JIT
    try:
        import jax, jax.numpy as jnp

        if _PACK_JIT is None:
            @jax.jit
            def pack(a):
                b = a.astype(jnp.bfloat16).reshape(-1, 2, 256, 4, 128)
                return jnp.transpose(b, (0, 4, 3, 1, 2))

            _PACK_JIT = pack
        with jax.default_device(jax.devices("cpu")[0]):
            return np.asarray(_PACK_JIT(x)).reshape(x.shape[0] // 2, 128, 2048)
    except Exception:
        import ml_dtypes

        b = x.astype(ml_dtypes.bfloat16).reshape(-1, 2, 256, 4, 128)
        return np.ascontiguousarray(b.transpose(0, 4, 3, 1, 2)).reshape(
            x.shape[0] // 2, 128, 2048
        )


def kernel(x, Wq, Wk, Wv, _nc=None, _trace=False):
    import ml_dtypes

    bf16 = ml_dtypes.bfloat16
    x = np.asarray(x)
    nb = x.shape[0] // N_CORES
    xb = _pack_x(x)  # [B//2, 128, 2048] bf16
    wqk = np.concatenate(
        [np.asarray(Wq, np.float32), np.asarray(Wk, np.float32)], axis=1
    )  # [512, 128]
    wqk = np.ascontiguousarray(wqk.reshape(4, 128, 128).transpose(1, 0, 2)).reshape(128, 512).astype(bf16)
    wv = np.ascontiguousarray(
        np.asarray(Wv, np.float32).reshape(4, 128, 64).transpose(1, 0, 2)
    ).reshape(128, 256).astype(bf16)

    nc = _nc if _nc is not None else _get_nc(nb)
    np2 = nb // 2
    in_maps = [
        {"x": xb[i * np2 : (i + 1) * np2], "Wqk": wqk, "Wv": wv}
        for i in range(N_CORES)
    ]
    res = run_bass_kernel_spmd(nc, in_maps, core_ids=list(range(N_CORES)), trace=_trace)
    outs = []
    for i in range(N_CORES):
        yd = np.asarray(res.results[i]["y"]).astype(np.float32)  # [nb//2, 128, 256]
        yd = yd.reshape(nb // 2, 128, 2, 2, 64).transpose(0, 2, 3, 1, 4).reshape(nb, 256, 64)
        outs.append(yd)
    out = np.ascontiguousarray(np.concatenate(outs, axis=0))
    if _trace:
        kernel.last_results = res
    return out


# revision 31
# speedup vs baseline: 1.0857x; 1.0857x over previous
"""Single-head causal attention (B=1024,T=256,C=512,H=64), data-parallel on 8 TRN2 cores.

Host side: cast x to bf16; pack Wq|Wk into a [128, 4*128] stationary layout
(chunk j at cols 128j, cols 0:64 = Wq chunk, 64:128 = Wk chunk) and Wv into
[128, 4*64]. Device output is [nb/8, 128, 1024] f32 in a permuted layout that
the host untangles (big contiguous HBM lines for the store DMA).

Device side, per pair of batches (b0, b0+1):
  xt [128, 2048] bf16 = xT chunks, layout j*512 + b*256 + t, filled by 8
      XBAR transpose-DMAs straight from HBM (no PE transposes, no casts)
  qkT [128 (q|k), 512 (b,t)] = Wqk^T @ xT   (4 matmuls, N=512)
  v   [128 (s), 4*64 (b,st)] natural, xT chunks stationary (16 small matmuls)
  v1  [128, 130] per batch = [v | 1] per s-tile (ones col -> softmax denom)
  weiT/exp: w_ps [128 (s), 384 (t-tile0 256 | diag1 128)], exp via one
      scalar activation, causal mask via 2 affine_selects on diagonal blocks
  out natural [t, 65] = sum_st e_chunk^T @ v1_st; normalize by col 64
"""

import sys, json

for _p in ("/opt/trn_rl_repo", "/root/.axon_site/_ro/trn_rl_repo"):
    if _p not in sys.path:
        sys.path.append(_p)

import numpy as np
import concourse.bass as bass
import concourse.tile as tile
from concourse import mybir
from concourse.bass_utils import run_bass_kernel_spmd

N_CORES = 8
B, T, C, H = 1024, 256, 512, 64
NB = B // N_CORES  # batches per core
CD = mybir.dt.bfloat16
F32 = mybir.dt.float32

_MAX_CTRL_WAITS = 1


def _patch_waits(nc):
    """walrus on this toolchain rejects >1 sync-wait on TPB_CTRL (NoOp/Drain/
    EventSemaphore) instructions; hoist excess waits into preceding NoOps."""
    raw = type(nc).to_json_bytes(nc)
    j = json.loads(raw)
    ctr = 0
    for f in j.get("functions", []):
        for bb in f.get("basicblocks", f.get("blocks", [])):
            out = []
            for i in bb.get("instructions", []):
                si = i.get("sync_info") or {}
                ow = si.get("on_wait") or []
                has_update = bool((si.get("on_update") or []))
                splittable = i.get("opcode") != "EventSemaphore" or not has_update
                if len(ow) > _MAX_CTRL_WAITS and splittable:
                    excess, keep = ow[:-_MAX_CTRL_WAITS], ow[-_MAX_CTRL_WAITS:]
                    while excess:
                        chunk, excess = excess[:_MAX_CTRL_WAITS], excess[_MAX_CTRL_WAITS:]
                        ctr += 1
                        out.append({
                            "name": f"WSPLIT-{ctr}",
                            "opcode": "NoOp",
                            "engine": i["engine"],
                            "ins": [], "outs": [],
                            "debug": i.get("debug", 0),
                            "sync_info": {"on_wait": chunk, "on_update": []},
                        })
                    si["on_wait"] = keep
                    i["sync_info"] = si
                out.append(i)
            bb["instructions"] = out
    data = json.dumps(j).encode()
    nc.to_json_bytes = lambda: data
    return nc


def build(nb=NB):
    assert nb % 2 == 0
    nc = bass.Bass("TRN2", target_bir_lowering=False, debug=False, enable_asserts=False)
    # host-packed xT: [pair, c-in-chunk, (j, b, t)] — 4KB contiguous HBM lines
    x = nc.dram_tensor("x", [nb // 2, 128, 2048], CD, kind="ExternalInput").ap()
    wqk_d = nc.dram_tensor("Wqk", [128, 512], CD, kind="ExternalInput").ap()
    wv_d = nc.dram_tensor("Wv", [128, 256], CD, kind="ExternalInput").ap()
    y = nc.dram_tensor("y", [nb // 2, 128, 256], CD, kind="ExternalOutput").ap()

    with tile.TileContext(nc) as tc:
        with (
            tc.tile_pool(name="consts", bufs=1) as consts,
            tc.tile_pool(name="xt", bufs=5) as p_xt,
            tc.tile_pool(name="qk", bufs=3) as p_qk,
            tc.tile_pool(name="v1", bufs=6) as p_v1,
            tc.tile_pool(name="e", bufs=6) as p_e,
            tc.tile_pool(name="rcp", bufs=4) as p_rcp,
            tc.tile_pool(name="go", bufs=4) as p_go,
            tc.tile_pool(name="ps", bufs=2, space="PSUM") as p_ps,
        ):
            wqk = consts.tile([128, 512], CD)
            nc.sync.dma_start(wqk[:], wqk_d[:, :])
            wvs = consts.tile([128, 256], CD)
            nc.sync.dma_start(wvs[:], wv_d[:, :])

            def emit_tail(prev):
                """out matmuls + normalize + store for an earlier pair (deps long ready)."""
                gout = p_go.tile([128, 256], CD, tag="gout", name="gout")
                for b in range(2):
                    e, v1 = prev["e"][b], prev["v1"][b]
                    o_ps = p_ps.tile([128, 130], F32, tag="o_ps", name="o_ps")
                    nc.tensor.matmul(
                        o_ps[:, 0:65], e[:, 0:128], v1[:, 0:65],
                        start=True, stop=True, skip_group_check=True,
                    )
                    nc.tensor.matmul(
                        o_ps[:, 65:130], e[:, 128:256], v1[:, 0:65],
                        start=True, stop=False, skip_group_check=True,
                    )
                    nc.tensor.matmul(
                        o_ps[:, 65:130], e[:, 256:384], v1[:, 65:130],
                        start=False, stop=True, skip_group_check=True,
                    )
                    rcp = p_rcp.tile([128, 2], F32, tag="rcp", name="rcp")
                    nc.vector.reciprocal(rcp[:, 0:1], o_ps[:, 64:65])
                    nc.vector.reciprocal(rcp[:, 1:2], o_ps[:, 129:130])
                    nc.scalar.mul(
                        gout[:, b * 128 : b * 128 + 64], o_ps[:, 0:64], rcp[:, 0:1]
                    )
                    nc.vector.tensor_scalar_mul(
                        gout[:, b * 128 + 64 : b * 128 + 128], o_ps[:, 65:129], rcp[:, 1:2]
                    )
                nc.sync.dma_start(y[prev["pair"]], gout[:])

            prev = None
            for pair in range(nb // 2):
                # ---- xT tile [128 (c-in-chunk), (j, b, t)] straight from HBM ----
                xt = p_xt.tile([128, 2048], CD, tag="xt", name="xt")
                nc.sync.dma_start(xt[:], x[pair])

                # ones columns for [v|1] blocks, written before the casts queue up
                v_ps = p_ps.tile([128, 260], F32, tag="v_ps", name="v_ps")
                for g in range(4):
                    nc.vector.memset(v_ps[:, g * 65 + 64 : g * 65 + 65], 1.0)

                # ---- qkT [128 (q0:64|k64:128), 512 (b,t)] ----
                qk_ps = p_ps.tile([128, 512], F32, tag="qk_ps", name="qk_ps")
                for j in range(4):
                    nc.tensor.matmul(
                        qk_ps[:],
                        wqk[:, 128 * j : 128 * (j + 1)],
                        xt[:, j * 512 : (j + 1) * 512],
                        start=(j == 0), stop=(j == 3),
                    )
                qt = p_qk.tile([64, 512], CD, tag="qt", name="qt")
                nc.vector.tensor_copy(qt[:], qk_ps[0:64, :])
                kt = p_qk.tile([64, 512], CD, tag="kt", name="kt")
                nc.vector.tensor_copy(kt[:], qk_ps[64:128, :])

                # ---- previous pair's output stage fills PE while exp/mask runs ----
                if prev is not None:
                    emit_tail(prev)

                # ---- v natural, pre-gapped [v|1] blocks: g at cols 65g ----
                for b in range(2):
                    for st in range(2):
                        g = b * 2 + st
                        for j in range(4):
                            nc.tensor.matmul(
                                v_ps[:, g * 65 : g * 65 + 64],
                                xt[:, j * 512 + b * 256 + st * 128 : j * 512 + b * 256 + st * 128 + 128],
                                wvs[:, 64 * j : 64 * (j + 1)],
                                start=(j == 0), stop=(j == 3),
                                skip_group_check=True,
                            )

                cur = {"pair": pair, "e": [], "v1": []}
                for b in range(2):
                    # ---- v1 [128, 130]: [v_s0 | 1 | v_s1 | 1], single copy ----
                    v1 = p_v1.tile([128, 130], CD, tag="v1", name="v1")
                    if b == 0:
                        nc.scalar.copy(v1[:], v_ps[:, 0:130])
                    else:
                        nc.vector.tensor_copy(v1[:], v_ps[:, 130:260])

                    # ---- weiT: [s0, t 0:256 | s1, t 128:256] ----
                    w_ps = p_ps.tile([128, 384], F32, tag="w_ps", name="w_ps")
                    nc.tensor.matmul(
                        w_ps[:, 0:256],
                        kt[:, b * 256 : b * 256 + 128],
                        qt[:, b * 256 : (b + 1) * 256],
                        start=True, stop=True, skip_group_check=True,
                    )
                    nc.tensor.matmul(
                        w_ps[:, 256:384],
                        kt[:, b * 256 + 128 : (b + 1) * 256],
                        qt[:, b * 256 + 128 : (b + 1) * 256],
                        start=True, stop=True, skip_group_check=True,
                    )
                    e = p_e.tile([128, 384], CD, tag="e", name="e")
                    nc.scalar.activation(e[:], w_ps[:], mybir.ActivationFunctionType.Exp, scale=0.125)
                    # causal mask only on the diagonal blocks (t>=s kept)
                    for off in (0, 256):
                        nc.gpsimd.affine_select(
                            out=e[:, off : off + 128], in_=e[:, off : off + 128],
                            compare_op=mybir.AluOpType.is_ge,
                            fill=0.0, base=0, pattern=[[1, 128]], channel_multiplier=-1,
                        )
                    cur["e"].append(e)
                    cur["v1"].append(v1)
                prev = cur

            emit_tail(prev)

    return _patch_waits(nc)


_CACHED = {}


def _get_nc(nb=NB):
    if nb not in _CACHED:
        _CACHED[nb] = build(nb)
    return _CACHED[nb]


_PACK_JIT = None


def _pack_x(x):
    """[B, 256, 512] f32 -> [B//2 pairs, 128, (j,b,t)] bf16, xT chunk layout."""
    global _PACK_JIT
    try:
        import jax, jax.numpy as jnp

        if _PACK_JIT is None:
            @jax.jit
            def pack(a):
                b = a.astype(jnp.bfloat16).reshape(-1, 2, 256, 4, 128)
                return jnp.transpose(b, (0, 4, 3, 1, 2))

            _PACK_JIT = pack
        with jax.default_device(jax.devices("cpu")[0]):
            return np.asarray(_PACK_JIT(x)).reshape(x.shape[0] // 2, 128, 2048)
    except Exception:
        import ml_dtypes

        b = x.astype(ml_dtypes.bfloat16).reshape(-1, 2, 256, 4, 128)
        return np.ascontiguousarray(b.transpose(0, 4, 3, 1, 2)).reshape(
            x.shape[0] // 2, 128, 2048
        )


def kernel(x, Wq, Wk, Wv, _nc=None, _trace=False):
    import ml_dtypes

    bf16 = ml_dtypes.bfloat16
    x = np.asarray(x)
    nb = x.shape[0] // N_CORES
    xb = _pack_x(x)  # [B//2, 128, 2048] bf16
    wqk = np.concatenate(
        [np.asarray(Wq, np.float32), np.asarray(Wk, np.float32)], axis=1
    )  # [512, 128]
    wqk = np.ascontiguousarray(wqk.reshape(4, 128, 128).transpose(1, 0, 2)).reshape(128, 512).astype(bf16)
    wv = np.ascontiguousarray(
        np.asarray(Wv, np.float32).reshape(4, 128, 64).transpose(1, 0, 2)
    ).reshape(128, 256).astype(bf16)

    nc = _nc if _nc is not None else _get_nc(nb)
    np2 = nb // 2
    in_maps = [
        {"x": xb[i * np2 : (i + 1) * np2], "Wqk": wqk, "Wv": wv}
        for i in range(N_CORES)
    ]
    res = run_bass_kernel_spmd(nc, in_maps, core_ids=list(range(N_CORES)), trace=_trace)
    outs = []
    for i in range(N_CORES):
        yd = np.asarray(res.results[i]["y"]).astype(np.float32)  # [nb//2, 128, 256]
        yd = yd.reshape(nb // 2, 128, 2, 2, 64).transpose(0, 2, 3, 1, 4).reshape(nb, 256, 64)
        outs.append(yd)
    out = np.ascontiguousarray(np.concatenate(outs, axis=0))
    if _trace:
        kernel.last_results = res
    return out


# revision 35
# speedup vs baseline: 1.1647x; 1.0727x over previous
"""Single-head causal attention (B=1024,T=256,C=512,H=64), data-parallel on 8 TRN2 cores.

Host side: cast x to bf16 AND pre-transpose into the xT chunk layout
[pair, c-in-chunk(128), (j, b, t)] via a jitted jax-cpu transform, so every
device load is a plain DMA with 4KB-contiguous HBM lines (the on-device
alternatives — PE transposes or XBAR transpose-DMAs — were the bottleneck
and, for XBAR, racy at scale). Pack Wq|Wk into a [128, 4*128] stationary
layout (chunk j at cols 128j; cols 0:64 = Wq chunk, 64:128 = Wk chunk) and
Wv into [128, 4*64]. Output comes back bf16 as [pair, t(128), (b, tt, h)];
the host casts to f32 and untangles.

Device side, per pair of batches, software-pipelined one pair deep:
  xt [128, 2048] bf16: one DMA per pair
  qkT [128 (q0:64|k64:128), 512 (b,t)] = Wqk^T @ xT  (4 matmuls, N=512)
  qt/kt [64, 512] bf16 casts (vector) — PSUM eviction for the wei stationaries
  v [128 (s), (b,st)*64] natural, xT chunks stationary (16 small matmuls);
  v1 [128, 130] = [v_s0 | 1 | v_s1 | 1] (ones cols -> softmax denominator),
      copies split scalar/vector
  weiT w_ps [128 (s), 384] = [s0 x t 0:256 | s1 x t 128:256]; exp in one
      scalar activation (scale=1/8); causal mask = 2 affine_selects on the
      128x128 diagonal blocks only
  out natural o_ps [t, 65|65] = e_chunk^T @ v1_st (3 matmuls); normalize via
      reciprocal of the denom cols, one scalar.mul + one vector tensor_scalar;
      the whole out stage runs one pair behind (emit_tail) so the PE never
      stalls on the exp/mask chain
"""

import sys, json

for _p in ("/opt/trn_rl_repo", "/root/.axon_site/_ro/trn_rl_repo"):
    if _p not in sys.path:
        sys.path.append(_p)

import numpy as np
import concourse.bass as bass
import concourse.tile as tile
from concourse import mybir
from concourse.bass_utils import run_bass_kernel_spmd

N_CORES = 8
B, T, C, H = 1024, 256, 512, 64
NB = B // N_CORES  # batches per core
CD = mybir.dt.bfloat16
F32 = mybir.dt.float32

_MAX_CTRL_WAITS = 1


def _patch_waits(nc):
    """walrus on this toolchain rejects >1 sync-wait on TPB_CTRL (NoOp/Drain/
    EventSemaphore) instructions; hoist excess waits into preceding NoOps."""
    raw = type(nc).to_json_bytes(nc)
    j = json.loads(raw)
    ctr = 0
    for f in j.get("functions", []):
        for bb in f.get("basicblocks", f.get("blocks", [])):
            out = []
            for i in bb.get("instructions", []):
                si = i.get("sync_info") or {}
                ow = si.get("on_wait") or []
                has_update = bool((si.get("on_update") or []))
                splittable = i.get("opcode") != "EventSemaphore" or not has_update
                if len(ow) > _MAX_CTRL_WAITS and splittable:
                    excess, keep = ow[:-_MAX_CTRL_WAITS], ow[-_MAX_CTRL_WAITS:]
                    while excess:
                        chunk, excess = excess[:_MAX_CTRL_WAITS], excess[_MAX_CTRL_WAITS:]
                        ctr += 1
                        out.append({
                            "name": f"WSPLIT-{ctr}",
                            "opcode": "NoOp",
                            "engine": i["engine"],
                            "ins": [], "outs": [],
                            "debug": i.get("debug", 0),
                            "sync_info": {"on_wait": chunk, "on_update": []},
                        })
                    si["on_wait"] = keep
                    i["sync_info"] = si
                out.append(i)
            bb["instructions"] = out
    data = json.dumps(j).encode()
    nc.to_json_bytes = lambda: data
    return nc


def build(nb=NB):
    assert nb % 2 == 0
    nc = bass.Bass("TRN2", target_bir_lowering=False, debug=False, enable_asserts=False)
    # host-packed xT: [pair, c-in-chunk, (j, b, t)] — 4KB contiguous HBM lines
    x = nc.dram_tensor("x", [nb // 2, 128, 2048], CD, kind="ExternalInput").ap()
    wqk_d = nc.dram_tensor("Wqk", [128, 512], CD, kind="ExternalInput").ap()
    wv_d = nc.dram_tensor("Wv", [128, 256], CD, kind="ExternalInput").ap()
    y = nc.dram_tensor("y", [nb // 2, 128, 256], CD, kind="ExternalOutput").ap()

    with tile.TileContext(nc) as tc:
        with (
            tc.tile_pool(name="consts", bufs=1) as consts,
            tc.tile_pool(name="xt", bufs=5) as p_xt,
            tc.tile_pool(name="qk", bufs=3) as p_qk,
            tc.tile_pool(name="v1", bufs=6) as p_v1,
            tc.tile_pool(name="e", bufs=6) as p_e,
            tc.tile_pool(name="rcp", bufs=4) as p_rcp,
            tc.tile_pool(name="go", bufs=4) as p_go,
            tc.tile_pool(name="ps", bufs=2, space="PSUM") as p_ps,
        ):
            wqk = consts.tile([128, 512], CD)
            nc.sync.dma_start(wqk[:], wqk_d[:, :])
            wvs = consts.tile([128, 256], CD)
            nc.sync.dma_start(wvs[:], wv_d[:, :])

            def emit_tail(prev):
                """out matmuls + normalize + store for an earlier pair (deps long ready)."""
                gout = p_go.tile([128, 256], CD, tag="gout", name="gout")
                for b in range(2):
                    e, v1 = prev["e"][b], prev["v1"][b]
                    o_ps = p_ps.tile([128, 130], F32, tag="o_ps", name="o_ps")
                    nc.tensor.matmul(
                        o_ps[:, 0:65], e[:, 0:128], v1[:, 0:65],
                        start=True, stop=True, skip_group_check=True,
                    )
                    nc.tensor.matmul(
                        o_ps[:, 65:130], e[:, 128:256], v1[:, 0:65],
                        start=True, stop=False, skip_group_check=True,
                    )
                    nc.tensor.matmul(
                        o_ps[:, 65:130], e[:, 256:384], v1[:, 65:130],
                        start=False, stop=True, skip_group_check=True,
                    )
                    rcp = p_rcp.tile([128, 2], F32, tag="rcp", name="rcp")
                    nc.vector.reciprocal(rcp[:, 0:1], o_ps[:, 64:65])
                    nc.vector.reciprocal(rcp[:, 1:2], o_ps[:, 129:130])
                    nc.scalar.mul(
                        gout[:, b * 128 : b * 128 + 64], o_ps[:, 0:64], rcp[:, 0:1]
                    )
                    nc.vector.tensor_scalar_mul(
                        gout[:, b * 128 + 64 : b * 128 + 128], o_ps[:, 65:129], rcp[:, 1:2]
                    )
                nc.sync.dma_start(y[prev["pair"]], gout[:])

            prev = None
            for pair in range(nb // 2):
                # ---- xT tile [128 (c-in-chunk), (j, b, t)] straight from HBM ----
                xt = p_xt.tile([128, 2048], CD, tag="xt", name="xt")
                nc.sync.dma_start(xt[:], x[pair])

                # ---- qkT [128 (q0:64|k64:128), 512 (b,t)] ----
                qk_ps = p_ps.tile([128, 512], F32, tag="qk_ps", name="qk_ps")
                for j in range(4):
                    nc.tensor.matmul(
                        qk_ps[:],
                        wqk[:, 128 * j : 128 * (j + 1)],
                        xt[:, j * 512 : (j + 1) * 512],
                        start=(j == 0), stop=(j == 3),
                    )
                qt = p_qk.tile([64, 512], CD, tag="qt", name="qt")
                nc.vector.tensor_copy(qt[:], qk_ps[0:64, :])
                kt = p_qk.tile([64, 512], CD, tag="kt", name="kt")
                nc.vector.tensor_copy(kt[:], qk_ps[64:128, :])

                # ---- previous pair's output stage fills PE while exp/mask runs ----
                if prev is not None:
                    emit_tail(prev)

                # ---- v natural [128 (s), (b, st)*64], xT chunks stationary ----
                v_ps = p_ps.tile([128, 256], F32, tag="v_ps", name="v_ps")
                for b in range(2):
                    for st in range(2):
                        g = b * 2 + st
                        for j in range(4):
                            nc.tensor.matmul(
                                v_ps[:, g * 64 : (g + 1) * 64],
                                xt[:, j * 512 + b * 256 + st * 128 : j * 512 + b * 256 + st * 128 + 128],
                                wvs[:, 64 * j : 64 * (j + 1)],
                                start=(j == 0), stop=(j == 3),
                                skip_group_check=True,
                            )

                cur = {"pair": pair, "e": [], "v1": []}
                for b in range(2):
                    # ---- v1 [128, 130]: [v_s0 | 1 | v_s1 | 1] ----
                    v1 = p_v1.tile([128, 130], CD, tag="v1", name="v1")
                    nc.gpsimd.memset(v1[:], 1.0)
                    if b == 0:
                        nc.scalar.copy(v1[:, 0:64], v_ps[:, 0:64])
                        nc.scalar.copy(v1[:, 65:129], v_ps[:, 64:128])
                    else:
                        nc.vector.tensor_copy(v1[:, 0:64], v_ps[:, 128:192])
                        nc.vector.tensor_copy(v1[:, 65:129], v_ps[:, 192:256])

                    # ---- weiT: [s0, t 0:256 | s1, t 128:256] ----
                    w_ps = p_ps.tile([128, 384], F32, tag="w_ps", name="w_ps")
                    nc.tensor.matmul(
                        w_ps[:, 0:256],
                        kt[:, b * 256 : b * 256 + 128],
                        qt[:, b * 256 : (b + 1) * 256],
                        start=True, stop=True, skip_group_check=True,
                    )
                    nc.tensor.matmul(
                        w_ps[:, 256:384],
                        kt[:, b * 256 + 128 : (b + 1) * 256],
                        qt[:, b * 256 + 128 : (b + 1) * 256],
                        start=True, stop=True, skip_group_check=True,
                    )
                    e = p_e.tile([128, 384], CD, tag="e", name="e")
                    nc.scalar.activation(e[:], w_ps[:], mybir.ActivationFunctionType.Exp, scale=0.125)
                    # causal mask only on the diagonal blocks (t>=s kept)
                    for off in (0, 256):
                        nc.gpsimd.affine_select(
                            out=e[:, off : off + 128], in_=e[:, off : off + 128],
                            compare_op=mybir.AluOpType.is_ge,
                            fill=0.0, base=0, pattern=[[1, 128]], channel_multiplier=-1,
                        )
                    cur["e"].append(e)
                    cur["v1"].append(v1)
                prev = cur

            emit_tail(prev)

    return _patch_waits(nc)


_CACHED = {}


def _get_nc(nb=NB):
    if nb not in _CACHED:
        _CACHED[nb] = build(nb)
    return _CACHED[nb]


_PACK_JIT = None


def _pack_x(x):
    """[B, 256, 512] f32 -> [B//2 pairs, 128, (j,b,t)] bf16, xT chunk layout."""
    global _PACK_JIT
    try:
        import jax, jax.numpy as jnp

        if _PACK_JIT is None:
            @jax.jit
            def pack(a):
                b = a.astype(jnp.bfloat16).reshape(-1, 2, 256, 4, 128)
                return jnp.transpose(b, (0, 4, 3, 1, 2))

            _PACK_JIT = pack
        with jax.default_device(jax.devices("cpu")[0]):
            return np.asarray(_PACK_JIT(x)).reshape(x.shape[0] // 2, 128, 2048)
    except Exception:
        import ml_dtypes

        b = x.astype(ml_dtypes.bfloat16).reshape(-1, 2, 256, 4, 128)
        return np.ascontiguousarray(b.transpose(0, 4, 3, 1, 2)).reshape(
            x.shape[0] // 2, 128, 2048
        )


def kernel(x, Wq, Wk, Wv, _nc=None, _trace=False):
    import ml_dtypes

    bf16 = ml_dtypes.bfloat16
    x = np.asarray(x)
    nb = x.shape[0] // N_CORES
    xb = _pack_x(x)  # [B//2, 128, 2048] bf16
    wqk = np.concatenate(
        [np.asarray(Wq, np.float32), np.asarray(Wk, np.float32)], axis=1
    )  # [512, 128]
    wqk = np.ascontiguousarray(wqk.reshape(4, 128, 128).transpose(1, 0, 2)).reshape(128, 512).astype(bf16)
    wv = np.ascontiguousarray(
        np.asarray(Wv, np.float32).reshape(4, 128, 64).transpose(1, 0, 2)
    ).reshape(128, 256).astype(bf16)

    nc = _nc if _nc is not None else _get_nc(nb)
    np2 = nb // 2
    in_maps = [
        {"x": xb[i * np2 : (i + 1) * np2], "Wqk": wqk, "Wv": wv}
        for i in range(N_CORES)
    ]
    res = run_bass_kernel_spmd(nc, in_maps, core_ids=list(range(N_CORES)), trace=_trace)
    outs = []
    for i in range(N_CORES):
        yd = np.asarray(res.results[i]["y"]).astype(np.float32)  # [nb//2, 128, 256]
        yd = yd.reshape(nb // 2, 128, 2, 2, 64).transpose(0, 2, 3, 1, 4).reshape(nb, 256, 64)
        outs.append(yd)
    out = np.ascontiguousarray(np.concatenate(outs, axis=0))
    if _trace:
        kernel.last_results = res
    return out
